# revision 30
# baseline (speedup 1.0000x reference)
"""Trainium2 Bass kernel: PreTrainedBilinearInterpolator (RGGB Bayer demosaic).

Contract: kernel(**inputs) takes the FULL inputs
    x    : (8, 1, 2048, 2048) float32
    w_R, w_B, w_GR, w_GB : (2, 1, 3, 3) float32 bilinear constants
and returns the FULL output (8, 3, 2048, 2048) float32.

Sharding: pure data parallelism -- image i goes to NeuronCore i.

Per-core algorithm (one 2048x2048 image, shipped reflect-padded to
2050x2050 so borders need no special handling):
  The stride-2 convs of the reference are algebraically a 3x3 stencil on the
  full-res image with reflect-101 borders.  With raw neighbor sums
      S_v[i,c] = x[i-1,c] + x[i+1,c]
      S_h[i,c] = x[i,c-1] + x[i,c+1]
      S_hv     = band(S_h)  (the 4 diagonal neighbors)
      S_g      = S_h + S_v  (the 4 cross neighbors)
  every output value is one of {x, ch*S_h, cv*S_v, cc*S_g, cd*S_hv} selected
  by (row parity, col parity).

  Row tiles of 128 loaded rows are DMA'd in *parity-grouped* layout
  (partitions 0..63 odd image rows, 64..127 even image rows), one DMA per
  parity group.  The cross-partition sums S_v = band@x and S_hv = band@S_h
  are single constant 128x128 0/1 band-matrix matmuls on the TensorEngine
  (fp32: exact, since only two 1.0-weighted terms contribute).  Everything
  downstream is per-partition work spread over VectorE/ScalarE/GPSIMD, and
  the final clip(0,1) is a provable no-op for inputs in [0,1).
"""

import sys
from contextlib import ExitStack

import numpy as np

if "/opt/trn_rl_repo" not in sys.path:
    sys.path.insert(0, "/opt/trn_rl_repo")

H = 2048
W = 2048
N_CORES = 8
STEP = 126  # output rows produced per 128-row loaded tile


def _wband() -> np.ndarray:
    """lhsT (K=128 in-partitions, M=128 out-partitions) for the vertical sum.

    Partition convention per tile starting at (even) output row R0:
      part j      (0..63)   = odd  image row R0-1+2j
      part 64+m   (0..63)   = even image row R0+2m
    Even out row R0+2m (at part 64+m, m<=62) sums odd parts m, m+1.
    Odd  out row R0-1+2j (at part j, 1<=j<=63) sums even parts 64+j-1, 64+j.
    """
    wb = np.zeros((128, 128), np.float32)
    for m in range(63):
        wb[m, 64 + m] = 1.0
        wb[m + 1, 64 + m] = 1.0
    for j in range(1, 64):
        wb[63 + j, j] = 1.0
        wb[64 + j, j] = 1.0
    return wb


def build_nc(height, width, coeffs, reps=1):
    """reps>1 repeats the whole pipeline (same data, same result) so test
    harnesses can measure device time as the wall-clock slope over reps."""
    import concourse.mybir as mybir
    import concourse.tile as tile
    from concourse import bacc

    f32 = mybir.dt.float32
    c_cross, c_diag, c_horiz, c_vert = coeffs
    half = width // 2
    mchunk = min(512, half)  # fp32 matmul moving-operand limit

    # Bacc (not raw Bass): its compile pass splits multi-sem waits into
    # EventSemaphore chains — walrus only allows 1 inline wait per inst.
    nc = bacc.Bacc("TRN2", target_bir_lowering=False, debug=False)
    # x arrives reflect-padded by 1 on all sides: [height+2, width+2]
    xp = nc.declare_dram_parameter("x", [height + 2, width + 2], f32, isOutput=False)
    out = nc.declare_dram_parameter("out", [3, height, width], f32, isOutput=True)
    wb_d = nc.inline_tensor(_wband(), name="wband")

    with tile.TileContext(nc) as tc:
        with ExitStack() as ctx:
            const = ctx.enter_context(tc.tile_pool(name="const", bufs=1))
            sb = ctx.enter_context(tc.tile_pool(name="sb", bufs=4))
            ps = ctx.enter_context(tc.tile_pool(name="ps", bufs=2, space="PSUM"))

            wb_t = const.tile([128, 128], f32)
            nc.sync.dma_start(out=wb_t[:], in_=wb_d[:])

            for R0 in [r for _ in range(reps) for r in range(0, height, STEP)]:
                nrows = min(STEP, height - R0)  # output rows (always even)
                npairs = nrows // 2
                nload = 63 if nrows == STEP else npairs

                # Parity-grouped loads: odd image rows (padded rows R0+2i) to
                # parts 0.., even image rows (padded R0+1+2i) to parts 64..
                xt = sb.tile([128, width + 2], f32, tag="xt")
                nl = nload + 1
                with tc.high_priority(offset=100):
                    nc.sync.dma_start(
                        out=xt[0:nl, :], in_=xp[R0 : R0 + 2 * nl - 1 : 2, :]
                    )
                    nc.sync.dma_start(
                        out=xt[64 : 64 + nl, :], in_=xp[R0 + 1 : R0 + 2 * nl : 2, :]
                    )
                if nl < 64:  # short last tile: fill unused partitions with
                    # valid data so full-group ops never read uninit SBUF
                    npad = 64 - nl
                    nc.sync.dma_start(
                        out=xt[nl : 64, :], in_=xp[1 : 1 + 2 * npad : 2, :]
                    )
                    nc.sync.dma_start(
                        out=xt[64 + nl : 128, :], in_=xp[0 : 2 * npad : 2, :]
                    )

                # S_h[c] = x[c-1] + x[c+1]  (image-indexed, one op, edges
                # inherited from the reflect padding)
                sh = sb.tile([128, width], f32, tag="sh")
                nc.vector.tensor_add(sh[:], xt[:, 0:width], xt[:, 2 : width + 2])

                rt = sb.tile([128, width], f32, tag="rt")
                gt = sb.tile([128, width], f32, tag="gt")
                bt = sb.tile([128, width], f32, tag="bt")

                # Compute-engine APs must start at partition 0/32/64/96, so
                # assembly runs on the full 64-part groups; the halo
                # partitions (0 and 127) compute garbage that is never stored.
                E = slice(64, 128)  # even image rows (used parts 64..126)
                O = slice(0, 64)    # odd image rows (used parts 1..63)
                fec = slice(0, width, 2)       # even image cols, full width
                foc = slice(1, width, 2)       # odd image cols
                fxec = slice(1, width + 1, 2)  # same cols in padded xt
                fxoc = slice(2, width + 2, 2)

                # Full-width assembly with no PSUM dependency:
                # even rows: R G R G ... (R sites even cols, GR odd cols)
                # odd rows:  G B G B ... (GB sites even cols, B odd cols)
                nc.gpsimd.tensor_copy(rt[E, fec], xt[E, fxec])
                nc.scalar.mul(rt[E, foc], sh[E, foc], c_horiz)
                nc.gpsimd.tensor_copy(gt[E, foc], xt[E, fxoc])
                nc.gpsimd.tensor_copy(gt[O, fec], xt[O, fxec])
                nc.scalar.mul(bt[O, fec], sh[O, fec], c_horiz)
                nc.gpsimd.tensor_copy(bt[O, foc], xt[O, fxoc])

                # PSUM stage in one-bank column chunks (bufs=4 per tag) so
                # PSUM frees incrementally and consecutive row-tiles overlap:
                # 2 tags x 4 bufs x 1 bank = the 8 PSUM banks.
                sgt = sb.tile([128, width], f32, tag="sg", bufs=2)
                for q0 in range(0, width, mchunk):
                    qw = mchunk
                    sv = ps.tile([128, qw], f32, tag="sv", bufs=4)
                    shv = ps.tile([128, qw], f32, tag="shv", bufs=4)
                    # Plain fp32 matmuls (4 cyc/row): float32r would be
                    # full-rate but requires fp32r-rounded (lossy) inputs.
                    nc.tensor.matmul(
                        sv[:],
                        wb_t[:],
                        xt[:, 1 + q0 : 1 + q0 + qw],
                        start=True,
                        stop=True,
                    )
                    nc.tensor.matmul(
                        shv[:],
                        wb_t[:],
                        sh[:, q0 : q0 + qw],
                        start=True,
                        stop=True,
                    )

                    # S_g = S_h + S_v for this chunk
                    nc.vector.tensor_add(
                        sgt[:, q0 : q0 + qw], sh[:, q0 : q0 + qw], sv[:]
                    )

                    ec = slice(q0, q0 + qw, 2)      # even image cols in chunk
                    oc = slice(q0 + 1, q0 + qw, 2)  # odd image cols in chunk
                    lec = slice(0, qw, 2)           # local to chunk tiles
                    loc = slice(1, qw, 2)

                    nc.vector.tensor_scalar_mul(bt[E, ec], shv[E, lec], c_diag)
                    nc.scalar.mul(bt[E, oc], sv[E, loc], c_vert)
                    nc.scalar.mul(rt[O, ec], sv[O, lec], c_vert)
                    nc.vector.tensor_scalar_mul(rt[O, oc], shv[O, loc], c_diag)

                # Full-width S_g-based assembly (G at R and B sites)
                nc.scalar.mul(gt[E, fec], sgt[E, fec], c_cross)
                nc.scalar.mul(gt[O, foc], sgt[O, foc], c_cross)

                # Stores go out on the ACT HWDGE ring (nc.scalar), loads on
                # the SP ring (nc.sync): separate FIFOs, so a store waiting
                # on assembly never blocks the next tile's load.
                for ch, t_ in ((0, rt), (1, gt), (2, bt)):
                    nc.sync.dma_start(
                        out=out[ch, R0 : R0 + 2 * npairs : 2, :],
                        in_=t_[64 : 64 + npairs, :],
                    )
                    nc.sync.dma_start(
                        out=out[ch, R0 + 1 : R0 + 2 * npairs : 2, :],
                        in_=t_[1 : 1 + npairs, :],
                    )
    nc.finalize()  # Bacc: runs compile() — reg alloc + sync-wait legalization
    return nc


_NC_CACHE = {}


def _get_nc(height, width, coeffs):
    key = (height, width, coeffs)
    if key not in _NC_CACHE:
        _NC_CACHE[key] = build_nc(height, width, coeffs)
    return _NC_CACHE[key]


_RUNNER_CACHE = {}


def _get_runner(height, width, coeffs):
    """Compiled SPMD runner (one jit/NEFF compile per process), so repeated
    kernel() calls do not recompile. Mirrors bass2jax.run_bass_via_pjrt."""
    key = (height, width, coeffs)
    if key in _RUNNER_CACHE:
        return _RUNNER_CACHE[key]

    import jax
    import concourse.mybir as mybir
    from concourse import bass2jax
    from concourse.bass2jax import _bass_exec_p, install_neuronx_cc_hook
    from jax.experimental.shard_map import shard_map
    from jax.sharding import Mesh, NamedSharding, PartitionSpec

    install_neuronx_cc_hook()
    nc = _get_nc(height, width, coeffs)

    partition_name = nc.partition_id_tensor.name if nc.partition_id_tensor else None
    in_names, out_names, out_avals = [], [], []
    for alloc in nc.m.functions[0].allocations:
        if not isinstance(alloc, mybir.MemoryLocationSet):
            continue
        name = alloc.memorylocations[0].name
        if alloc.kind == "ExternalInput":
            if name != partition_name:
                in_names.append(name)
        elif alloc.kind == "ExternalOutput":
            out_names.append(name)
            out_avals.append(
                jax.core.ShapedArray(
                    tuple(alloc.tensor_shape), mybir.dt.np(alloc.dtype)
                )
            )
    assert in_names == ["x"] and out_names == ["out"], (in_names, out_names)
    n_params = len(in_names)
    all_in = in_names + out_names + ([partition_name] if partition_name else [])

    def _body(*args):
        operands = list(args)
        if partition_name is not None:
            operands.append(bass2jax.partition_id_tensor())
        return tuple(
            _bass_exec_p.bind(
                *operands,
                out_avals=tuple(out_avals),
                in_names=tuple(all_in),
                out_names=tuple(out_names),
                lowering_input_output_aliases=(),
                sim_require_finite=True,
                sim_require_nnan=True,
                nc=nc,
            )
        )

    devices = jax.devices()[:N_CORES]
    assert len(devices) == N_CORES, devices
    mesh = Mesh(np.asarray(devices), ("core",))
    n_outs = len(out_names)
    f = jax.jit(
        shard_map(
            _body,
            mesh=mesh,
            in_specs=(PartitionSpec("core"),) * (n_params + n_outs),
            out_specs=(PartitionSpec("core"),) * n_outs,
            check_rep=False,
        ),
        keep_unused=True,
    )
    shard = NamedSharding(mesh, PartitionSpec("core"))
    zeros = [
        jax.device_put(
            np.zeros((N_CORES * a.shape[0], *a.shape[1:]), a.dtype), shard
        )
        for a in out_avals
    ]
    jax.block_until_ready(zeros)

    def runner(x_pad_concat: np.ndarray) -> np.ndarray:
        xd = jax.device_put(x_pad_concat, shard)
        (out_arr,) = f(xd, *zeros)
        res = np.asarray(out_arr).reshape(N_CORES, *out_avals[0].shape)
        return res

    _RUNNER_CACHE[key] = runner
    return runner


def _coeffs(w_R, w_GR):
    w_R = np.asarray(w_R)
    w_GR = np.asarray(w_GR)
    c_cross = float(w_R[0, 0, 0, 1])   # G at R/B sites: 4-neighbor cross
    c_diag = float(w_R[1, 0, 0, 0])    # B/R at B/R sites: 4-neighbor diagonal
    c_horiz = float(w_GR[0, 0, 1, 0])  # left/right pair
    c_vert = float(w_GR[1, 0, 0, 1])   # up/down pair
    return (c_cross, c_diag, c_horiz, c_vert)


def kernel(x, w_R, w_B, w_GR, w_GB):
    x = np.asarray(x)
    n, _, height, width = x.shape
    assert (n, height, width) == (N_CORES, H, W), x.shape

    runner = _get_runner(height, width, _coeffs(w_R, w_GR))
    x_pad = np.concatenate(
        [
            np.pad(np.asarray(x[i, 0], dtype=np.float32), 1, mode="reflect")
            for i in range(N_CORES)
        ],
        axis=0,
    )
    return runner(x_pad).astype(np.float32)


# revision 32
# speedup vs baseline: 69.3587x; 69.3587x over previous
"""Trainium2 Bass kernel: PreTrainedBilinearInterpolator (RGGB Bayer demosaic).

Contract: kernel(**inputs) takes the FULL inputs
    x    : (8, 1, 2048, 2048) float32
    w_R, w_B, w_GR, w_GB : (2, 1, 3, 3) float32 bilinear constants
and returns the FULL output (8, 3, 2048, 2048) float32.

Sharding: pure data parallelism -- image i goes to NeuronCore i.

Per-core algorithm (one 2048x2048 image, shipped reflect-padded to
2050x2050 so borders need no special handling):
  The stride-2 convs of the reference are algebraically a 3x3 stencil on the
  full-res image with reflect-101 borders.  With raw neighbor sums
      S_v[i,c] = x[i-1,c] + x[i+1,c]
      S_h[i,c] = x[i,c-1] + x[i,c+1]
      S_hv     = band(S_h)  (the 4 diagonal neighbors)
      S_g      = S_h + S_v  (the 4 cross neighbors)
  every output value is one of {x, ch*S_h, cv*S_v, cc*S_g, cd*S_hv} selected
  by (row parity, col parity).

  Row tiles of 128 loaded rows are DMA'd in *parity-grouped* layout
  (partitions 0..63 odd image rows, 64..127 even image rows), one DMA per
  parity group.  The cross-partition sums S_v = band@x and S_hv = band@S_h
  are single constant 128x128 0/1 band-matrix matmuls on the TensorEngine
  (fp32: exact, since only two 1.0-weighted terms contribute).  Everything
  downstream is per-partition work spread over VectorE/ScalarE/GPSIMD, and
  the final clip(0,1) is a provable no-op for inputs in [0,1).
"""

import sys
from contextlib import ExitStack

import numpy as np

if "/opt/trn_rl_repo" not in sys.path:
    sys.path.insert(0, "/opt/trn_rl_repo")

H = 2048
W = 2048
N_CORES = 8
STEP = 126  # output rows produced per 128-row loaded tile


def _wband() -> np.ndarray:
    """lhsT (K=128 in-partitions, M=128 out-partitions) for the vertical sum.

    Partition convention per tile starting at (even) output row R0:
      part j      (0..63)   = odd  image row R0-1+2j
      part 64+m   (0..63)   = even image row R0+2m
    Even out row R0+2m (at part 64+m, m<=62) sums odd parts m, m+1.
    Odd  out row R0-1+2j (at part j, 1<=j<=63) sums even parts 64+j-1, 64+j.
    """
    wb = np.zeros((128, 128), np.float32)
    for m in range(63):
        wb[m, 64 + m] = 1.0
        wb[m + 1, 64 + m] = 1.0
    for j in range(1, 64):
        wb[63 + j, j] = 1.0
        wb[64 + j, j] = 1.0
    return wb


def build_nc(height, width, coeffs, reps=1):
    """reps>1 repeats the whole pipeline (same data, same result) so test
    harnesses can measure device time as the wall-clock slope over reps."""
    import concourse.mybir as mybir
    import concourse.tile as tile
    from concourse import bacc

    f32 = mybir.dt.float32
    c_cross, c_diag, c_horiz, c_vert = coeffs
    half = width // 2
    mchunk = min(512, half)  # fp32 matmul moving-operand limit

    # Bacc (not raw Bass): its compile pass splits multi-sem waits into
    # EventSemaphore chains — walrus only allows 1 inline wait per inst.
    nc = bacc.Bacc("TRN2", target_bir_lowering=False, debug=False)
    # x arrives reflect-padded by 1 on all sides: [height+2, width+2]
    xp = nc.declare_dram_parameter("x", [height + 2, width + 2], f32, isOutput=False)
    out = nc.declare_dram_parameter("out", [3, height, width], f32, isOutput=True)
    wb_d = nc.inline_tensor(_wband(), name="wband")

    with tile.TileContext(nc) as tc:
        with ExitStack() as ctx:
            const = ctx.enter_context(tc.tile_pool(name="const", bufs=1))
            sb = ctx.enter_context(tc.tile_pool(name="sb", bufs=4))
            ps = ctx.enter_context(tc.tile_pool(name="ps", bufs=2, space="PSUM"))

            wb_t = const.tile([128, 128], f32)
            nc.sync.dma_start(out=wb_t[:], in_=wb_d[:])

            for R0 in [r for _ in range(reps) for r in range(0, height, STEP)]:
                nrows = min(STEP, height - R0)  # output rows (always even)
                npairs = nrows // 2
                nload = 63 if nrows == STEP else npairs

                # Parity-grouped loads: odd image rows (padded rows R0+2i) to
                # parts 0.., even image rows (padded R0+1+2i) to parts 64..
                xt = sb.tile([128, width + 2], f32, tag="xt")
                nl = nload + 1
                with tc.high_priority(offset=100):
                    nc.sync.dma_start(
                        out=xt[0:nl, :], in_=xp[R0 : R0 + 2 * nl - 1 : 2, :]
                    )
                    nc.sync.dma_start(
                        out=xt[64 : 64 + nl, :], in_=xp[R0 + 1 : R0 + 2 * nl : 2, :]
                    )
                if nl < 64:  # short last tile: fill unused partitions with
                    # valid data so full-group ops never read uninit SBUF
                    npad = 64 - nl
                    nc.sync.dma_start(
                        out=xt[nl : 64, :], in_=xp[1 : 1 + 2 * npad : 2, :]
                    )
                    nc.sync.dma_start(
                        out=xt[64 + nl : 128, :], in_=xp[0 : 2 * npad : 2, :]
                    )

                # S_h[c] = x[c-1] + x[c+1]  (image-indexed, one op, edges
                # inherited from the reflect padding)
                sh = sb.tile([128, width], f32, tag="sh")
                nc.vector.tensor_add(sh[:], xt[:, 0:width], xt[:, 2 : width + 2])

                rt = sb.tile([128, width], f32, tag="rt")
                gt = sb.tile([128, width], f32, tag="gt")
                bt = sb.tile([128, width], f32, tag="bt")

                # Compute-engine APs must start at partition 0/32/64/96, so
                # assembly runs on the full 64-part groups; the halo
                # partitions (0 and 127) compute garbage that is never stored.
                E = slice(64, 128)  # even image rows (used parts 64..126)
                O = slice(0, 64)    # odd image rows (used parts 1..63)
                fec = slice(0, width, 2)       # even image cols, full width
                foc = slice(1, width, 2)       # odd image cols
                fxec = slice(1, width + 1, 2)  # same cols in padded xt
                fxoc = slice(2, width + 2, 2)

                # Full-width assembly with no PSUM dependency:
                # even rows: R G R G ... (R sites even cols, GR odd cols)
                # odd rows:  G B G B ... (GB sites even cols, B odd cols)
                nc.gpsimd.tensor_copy(rt[E, fec], xt[E, fxec])
                nc.scalar.mul(rt[E, foc], sh[E, foc], c_horiz)
                nc.gpsimd.tensor_copy(gt[E, foc], xt[E, fxoc])
                nc.gpsimd.tensor_copy(gt[O, fec], xt[O, fxec])
                nc.scalar.mul(bt[O, fec], sh[O, fec], c_horiz)
                nc.gpsimd.tensor_copy(bt[O, foc], xt[O, fxoc])

                # PSUM stage in one-bank column chunks (bufs=4 per tag) so
                # PSUM frees incrementally and consecutive row-tiles overlap:
                # 2 tags x 4 bufs x 1 bank = the 8 PSUM banks.
                sgt = sb.tile([128, width], f32, tag="sg", bufs=2)
                for q0 in range(0, width, mchunk):
                    qw = mchunk
                    sv = ps.tile([128, qw], f32, tag="sv", bufs=4)
                    shv = ps.tile([128, qw], f32, tag="shv", bufs=4)
                    # Plain fp32 matmuls (4 cyc/row): float32r would be
                    # full-rate but requires fp32r-rounded (lossy) inputs.
                    nc.tensor.matmul(
                        sv[:],
                        wb_t[:],
                        xt[:, 1 + q0 : 1 + q0 + qw],
                        start=True,
                        stop=True,
                    )
                    nc.tensor.matmul(
                        shv[:],
                        wb_t[:],
                        sh[:, q0 : q0 + qw],
                        start=True,
                        stop=True,
                    )

                    # S_g = S_h + S_v for this chunk
                    nc.vector.tensor_add(
                        sgt[:, q0 : q0 + qw], sh[:, q0 : q0 + qw], sv[:]
                    )

                    ec = slice(q0, q0 + qw, 2)      # even image cols in chunk
                    oc = slice(q0 + 1, q0 + qw, 2)  # odd image cols in chunk
                    lec = slice(0, qw, 2)           # local to chunk tiles
                    loc = slice(1, qw, 2)

                    nc.vector.tensor_scalar_mul(bt[E, ec], shv[E, lec], c_diag)
                    nc.scalar.mul(bt[E, oc], sv[E, loc], c_vert)
                    nc.scalar.mul(rt[O, ec], sv[O, lec], c_vert)
                    nc.vector.tensor_scalar_mul(rt[O, oc], shv[O, loc], c_diag)

                # Full-width S_g-based assembly (G at R and B sites)
                nc.scalar.mul(gt[E, fec], sgt[E, fec], c_cross)
                nc.scalar.mul(gt[O, foc], sgt[O, foc], c_cross)

                # Stores go out on the ACT HWDGE ring (nc.scalar), loads on
                # the SP ring (nc.sync): separate FIFOs, so a store waiting
                # on assembly never blocks the next tile's load.
                for ch, t_ in ((0, rt), (1, gt), (2, bt)):
                    nc.sync.dma_start(
                        out=out[ch, R0 : R0 + 2 * npairs : 2, :],
                        in_=t_[64 : 64 + npairs, :],
                    )
                    nc.sync.dma_start(
                        out=out[ch, R0 + 1 : R0 + 2 * npairs : 2, :],
                        in_=t_[1 : 1 + npairs, :],
                    )
    nc.finalize()  # Bacc: runs compile() — reg alloc + sync-wait legalization
    return nc


_NC_CACHE = {}


def _get_nc(height, width, coeffs):
    key = (height, width, coeffs)
    if key not in _NC_CACHE:
        _NC_CACHE[key] = build_nc(height, width, coeffs)
    return _NC_CACHE[key]


_RUNNER_CACHE = {}


def _get_runner(height, width, coeffs):
    """Compiled SPMD runner (one jit/NEFF compile per process), so repeated
    kernel() calls do not recompile. Mirrors bass2jax.run_bass_via_pjrt."""
    key = (height, width, coeffs)
    if key in _RUNNER_CACHE:
        return _RUNNER_CACHE[key]

    import jax
    import concourse.mybir as mybir
    from concourse import bass2jax
    from concourse.bass2jax import _bass_exec_p, install_neuronx_cc_hook
    from jax.experimental.shard_map import shard_map
    from jax.sharding import Mesh, NamedSharding, PartitionSpec

    install_neuronx_cc_hook()
    nc = _get_nc(height, width, coeffs)

    partition_name = nc.partition_id_tensor.name if nc.partition_id_tensor else None
    in_names, out_names, out_avals = [], [], []
    for alloc in nc.m.functions[0].allocations:
        if not isinstance(alloc, mybir.MemoryLocationSet):
            continue
        name = alloc.memorylocations[0].name
        if alloc.kind == "ExternalInput":
            if name != partition_name:
                in_names.append(name)
        elif alloc.kind == "ExternalOutput":
            out_names.append(name)
            out_avals.append(
                jax.core.ShapedArray(
                    tuple(alloc.tensor_shape), mybir.dt.np(alloc.dtype)
                )
            )
    assert in_names == ["x"] and out_names == ["out"], (in_names, out_names)
    n_params = len(in_names)
    all_in = in_names + out_names + ([partition_name] if partition_name else [])

    def _body(*args):
        operands = list(args)
        if partition_name is not None:
            operands.append(bass2jax.partition_id_tensor())
        return tuple(
            _bass_exec_p.bind(
                *operands,
                out_avals=tuple(out_avals),
                in_names=tuple(all_in),
                out_names=tuple(out_names),
                lowering_input_output_aliases=(),
                sim_require_finite=True,
                sim_require_nnan=True,
                nc=nc,
            )
        )

    devices = jax.devices()[:N_CORES]
    assert len(devices) == N_CORES, devices
    mesh = Mesh(np.asarray(devices), ("core",))
    n_outs = len(out_names)
    f = jax.jit(
        shard_map(
            _body,
            mesh=mesh,
            in_specs=(PartitionSpec("core"),) * (n_params + n_outs),
            out_specs=(PartitionSpec("core"),) * n_outs,
            check_rep=False,
        ),
        keep_unused=True,
    )
    shard = NamedSharding(mesh, PartitionSpec("core"))
    zeros = [
        jax.device_put(
            np.zeros((N_CORES * a.shape[0], *a.shape[1:]), a.dtype), shard
        )
        for a in out_avals
    ]
    jax.block_until_ready(zeros)

    def runner(x_pad_concat: np.ndarray) -> np.ndarray:
        xd = jax.device_put(x_pad_concat, shard)
        (out_arr,) = f(xd, *zeros)
        res = np.asarray(out_arr).reshape(N_CORES, *out_avals[0].shape)
        return res

    _RUNNER_CACHE[key] = runner
    return runner


def _coeffs(w_R, w_GR):
    w_R = np.asarray(w_R)
    w_GR = np.asarray(w_GR)
    c_cross = float(w_R[0, 0, 0, 1])   # G at R/B sites: 4-neighbor cross
    c_diag = float(w_R[1, 0, 0, 0])    # B/R at B/R sites: 4-neighbor diagonal
    c_horiz = float(w_GR[0, 0, 1, 0])  # left/right pair
    c_vert = float(w_GR[1, 0, 0, 1])   # up/down pair
    return (c_cross, c_diag, c_horiz, c_vert)


def kernel(x, w_R, w_B, w_GR, w_GB):
    x = np.asarray(x)
    n, _, height, width = x.shape
    assert (n, height, width) == (N_CORES, H, W), x.shape

    runner = _get_runner(height, width, _coeffs(w_R, w_GR))
    x_pad = np.concatenate(
        [
            np.pad(np.asarray(x[i, 0], dtype=np.float32), 1, mode="reflect")
            for i in range(N_CORES)
        ],
        axis=0,
    )
    return runner(x_pad).astype(np.float32)
